# revision 1
# baseline (speedup 1.0000x reference)
"""Trainium2 Bass kernel for nn_Agent_214748364878 (sparse_attention).

Strategy: pure data parallel over batch B=64 -> 8 batches per core.
The reference materializes huge [H,B,M,N,KS] glimpse tensors; instead we use
the algebraic identity  Q . (Kstat + ndf @ Wk)  =  Q . Kstat + (Q @ Wk) . ndf
so every big tensor is streamed exactly once through small block-diagonal
matmuls on the PE.

Shapes: B=64, M=5 vehicles, N=1000 nodes, D=128, H=8 heads, KS=16.
Output: softmax probs [64, 5000] (joint softmax over M*N per batch).
"""

import math
import numpy as np

B, M, N, D, H = 64, 5, 1000, 128, 8
KS = D // H
NCORES = 8
BL = B // NCORES        # 8 batches per core
NPAD = 1024             # N padded to 8 chunks of 128
NCHUNK = 8
MF = M * 8              # 40 = (m, feature) pairs
HM = H * M              # 40 = (head, vehicle) rows
MASKVAL = -1.0e6        # log(0) stand-in; exp() underflows to exactly 0.0

_CACHE = {}


def _build_program():
    import concourse.bass as bass
    import concourse.bacc as bacc
    import concourse.tile as tile
    from concourse import mybir

    f32 = mybir.dt.float32
    nc = bacc.Bacc("TRN2", target_bir_lowering=False, debug=False)

    # ---- DRAM tensors (per-core inputs, host-prearranged layouts) ----
    d_kst = nc.dram_tensor("kst", [BL, 128, NPAD], f32, kind="ExternalInput")
    d_lkst = nc.dram_tensor("lkst", [BL, 128, N], f32, kind="ExternalInput")
    d_vst = nc.dram_tensor("vst", [BL, NCHUNK, 128, 128], f32, kind="ExternalInput")
    d_ndft = nc.dram_tensor("ndft", [BL, MF + M, NPAD], f32, kind="ExternalInput")
    d_ndfn = nc.dram_tensor("ndfn", [BL, NCHUNK, 128, MF + 1], f32, kind="ExternalInput")
    d_prevT = nc.dram_tensor("prevT", [BL, 128, M], f32, kind="ExternalInput")
    d_vehT = nc.dram_tensor("vehT", [BL, 3, M], f32, kind="ExternalInput")
    d_fc = nc.dram_tensor("fc", [BL, 128, 1], f32, kind="ExternalInput")
    # constants (same on all cores)
    d_wpcvA = nc.dram_tensor("wpcvA", [128, 128], f32, kind="ExternalInput")
    d_wpcvB = nc.dram_tensor("wpcvB", [3, 128], f32, kind="ExternalInput")
    d_wk8 = nc.dram_tensor("wk8", [128, 8], f32, kind="ExternalInput")
    d_wl8 = nc.dram_tensor("wl8", [128, 8], f32, kind="ExternalInput")
    d_wvstk = nc.dram_tensor("wvstk", [MF, 128], f32, kind="ExternalInput")
    d_r8 = nc.dram_tensor("r8", [8, MF], f32, kind="ExternalInput")
    d_maskM = nc.dram_tensor("maskM", [MF, HM], f32, kind="ExternalInput")
    d_mask5 = nc.dram_tensor("mask5", [M, HM], f32, kind="ExternalInput")
    d_selT = nc.dram_tensor("selT", [HM, 128], f32, kind="ExternalInput")
    d_poT = nc.dram_tensor("poT", [128, 128], f32, kind="ExternalInput")
    d_ident = nc.dram_tensor("ident", [128, 128], f32, kind="ExternalInput")
    d_ones5 = nc.dram_tensor("ones5", [M, 1], f32, kind="ExternalInput")
    d_blkmask = nc.dram_tensor("blkmask", [128, HM], f32, kind="ExternalInput")
    d_ones15 = nc.dram_tensor("ones15", [1, M], f32, kind="ExternalInput")

    d_out = nc.dram_tensor("out", [BL, M, N], f32, kind="ExternalOutput")

    add = mybir.AluOpType.add
    mult = mybir.AluOpType.mult
    EXP = mybir.ActivationFunctionType.Exp
    TANH = mybir.ActivationFunctionType.Tanh

    with tile.TileContext(nc) as tc:
        with (
            tc.tile_pool(name="consts", bufs=1) as consts,
            tc.tile_pool(name="persist", bufs=1) as persist,
            tc.tile_pool(name="big", bufs=2) as big,
            tc.tile_pool(name="mid", bufs=2) as mid,
            tc.tile_pool(name="small", bufs=2) as small,
            tc.tile_pool(name="ps_mm", bufs=2, space="PSUM") as ps_mm,
            tc.tile_pool(name="ps_u", bufs=1, space="PSUM") as ps_u,
            tc.tile_pool(name="ps_s", bufs=1, space="PSUM") as ps_s,
            tc.tile_pool(name="ps_sm", bufs=2, space="PSUM") as ps_sm,
        ):
            # ---- load constants once ----
            def cload(dram, shape, tag):
                t = consts.tile(shape, f32, tag=tag)
                nc.sync.dma_start(t[:], dram.ap())
                return t

            wpcvA = cload(d_wpcvA, [128, 128], "c_wpcvA")
            wpcvB = cload(d_wpcvB, [3, 128], "c_wpcvB")
            wk8 = cload(d_wk8, [128, 8], "c_wk8")
            wl8 = cload(d_wl8, [128, 8], "c_wl8")
            wvstk = cload(d_wvstk, [MF, 128], "c_wvstk")
            r8 = cload(d_r8, [8, MF], "c_r8")
            maskM = cload(d_maskM, [MF, HM], "c_maskM")
            selT = cload(d_selT, [HM, 128], "c_selT")
            poT = cload(d_poT, [128, 128], "c_poT")
            ident = cload(d_ident, [128, 128], "c_ident")
            ones5 = cload(d_ones5, [M, 1], "c_ones5")
            ones15 = cload(d_ones15, [1, M], "c_ones15")
            blkmask = cload(d_blkmask, [128, HM], "c_blkmask")

            # persistent block-diagonal operand tiles
            lhsT1 = persist.tile([128, HM], f32)       # block-diag Q (scaled)
            lhsT2 = persist.tile([MF + M, HM], f32)    # block-diag QWk + mask rows
            lhsT3 = persist.tile([MF + M, M], f32)     # block-diag FWl + ident rows
            nc.vector.memset(lhsT1[:], 0.0)
            nc.sync.dma_start(lhsT2[MF:MF + M, :], d_mask5.ap())
            nc.sync.dma_start(lhsT3[MF:MF + M, :], d_mask5.ap()[:, 0:M])

            HALVES = [(0, 512), (512, 488)]

            for b in range(BL):
                # ---- stream per-batch inputs ----
                kst = big.tile([128, NPAD], f32, tag="kst")
                nc.sync.dma_start(kst[:], d_kst.ap()[b])
                lkst = big.tile([128, N], f32, tag="lkst")
                nc.sync.dma_start(lkst[:], d_lkst.ap()[b])
                vst = big.tile([128, NCHUNK, 128], f32, tag="vst")
                nc.sync.dma_start(vst[:], d_vst.ap()[b].rearrange("c p k -> p c k"))
                ndft = mid.tile([MF + M, NPAD], f32, tag="ndft")
                nc.sync.dma_start(ndft[:], d_ndft.ap()[b])
                ndfn = mid.tile([128, NCHUNK, MF + 1], f32, tag="ndfn")
                nc.sync.dma_start(ndfn[:], d_ndfn.ap()[b].rearrange("c p k -> p c k"))
                prevT = small.tile([128, M], f32, tag="prevT")
                nc.sync.dma_start(prevT[:], d_prevT.ap()[b])
                vehT = small.tile([3, M], f32, tag="vehT")
                nc.sync.dma_start(vehT[:], d_vehT.ap()[b])
                fc = small.tile([128, 1], f32, tag="fc")
                nc.sync.dma_start(fc[:], d_fc.ap()[b])
                maskb5 = mid.tile([M, N], f32, tag="maskb5")
                nc.sync.dma_start(maskb5[:], d_ndft.ap()[b, MF:MF + M, 0:N])

                # ---- query: qT[d, m] = W_pcv @ cvs.T + fc ----
                ps_q = ps_sm.tile([128, M], f32, tag="sm")
                nc.tensor.matmul(ps_q[:], wpcvA[:], prevT[:], start=True, stop=False)
                nc.tensor.matmul(ps_q[:], wpcvB[:], vehT[:], start=False, stop=True)
                qT = small.tile([128, M], f32, tag="qT")
                nc.vector.tensor_scalar_add(qT[:], ps_q[:], fc[:])

                # scatter into block-diag lhsT1: broadcast qT along h, mask
                nc.vector.tensor_mul(
                    lhsT1[:].rearrange("p (h m) -> p h m", h=H),
                    qT[:, None, :].broadcast_to([128, H, M]),
                    blkmask[:].rearrange("p (h m) -> p h m", h=H),
                )

                # ---- QWk[h,m,f] via wk8.T @ lhsT1 -> [8, 40] ----
                ps_qwk = ps_sm.tile([8, HM], f32, tag="sm")
                nc.tensor.matmul(ps_qwk[:], wk8[:], lhsT1[:])
                qwk = small.tile([8, HM], f32, tag="qwks")
                nc.vector.tensor_copy(qwk[:], ps_qwk[:])
                # replicate across m' (r8.T @ qwk -> [40, 40]) then mask
                ps_rep = ps_sm.tile([MF, HM], f32, tag="sm")
                nc.tensor.matmul(ps_rep[:], r8[:], qwk[:])
                nc.vector.tensor_mul(lhsT2[0:MF, :], ps_rep[:], maskM[:])

                # ---- compatT per n-chunk + exp + PV + S (no transposes) ----
                ET = mid.tile([128, NCHUNK * HM], f32, tag="ET")
                ps_U = ps_u.tile([HM, 128], f32, tag="U")
                ps_S = ps_s.tile([HM, MF + 1], f32, tag="S")
                for c in range(NCHUNK):
                    ps_ct = ps_mm.tile([128, HM], f32, tag="ct")
                    nc.tensor.matmul(ps_ct[:], kst[:, c * 128:(c + 1) * 128],
                                     lhsT1[:], start=True, stop=False)
                    nc.tensor.matmul(ps_ct[:], ndft[:, c * 128:(c + 1) * 128],
                                     lhsT2[:], start=False, stop=True)
                    nc.scalar.activation(ET[:, c * HM:(c + 1) * HM], ps_ct[:],
                                         EXP, scale=0.25)
                    nc.tensor.matmul(ps_U[:], ET[:, c * HM:(c + 1) * HM],
                                     vst[:, c, :], start=(c == 0), stop=False)
                    nc.tensor.matmul(ps_S[:], ET[:, c * HM:(c + 1) * HM],
                                     ndfn[:, c, :], start=(c == 0),
                                     stop=(c == NCHUNK - 1))
                # row sums came along as ndfn's ones column -> S[:, 40]
                r40 = small.tile([HM, 1], f32, tag="r40")
                nc.vector.tensor_copy(r40[:], ps_S[:, MF:MF + 1])

                # Z per head broadcast to (h*16+k) partitions, then 1/Z
                ps_z = ps_sm.tile([128, 1], f32, tag="sm")
                nc.tensor.matmul(ps_z[:], selT[:], r40[:])
                zbc = small.tile([128, 1], f32, tag="zbc")
                nc.vector.tensor_copy(zbc[:], ps_z[:])
                zinv = small.tile([128, 1], f32, tag="zinv")
                nc.vector.reciprocal(zinv[:], zbc[:])

                # ---- U2 = masked(S.T) @ Wv_stack accumulated into U ----
                S_sb = small.tile([HM, MF], f32, tag="S_sb")
                nc.vector.tensor_copy(S_sb[:], ps_S[:, 0:MF])
                ps_ST = ps_sm.tile([MF, HM], f32, tag="sm")
                nc.tensor.transpose(ps_ST[:], S_sb[:], ident[:HM, :HM])
                SmT = small.tile([MF, HM], f32, tag="SmT")
                nc.vector.tensor_mul(SmT[:], ps_ST[:], maskM[:])
                nc.tensor.matmul(ps_U[:], SmT[:], wvstk[:], start=False, stop=True)

                # ---- heads -> concatT (normalize by 1/Z) ----
                U_sb = small.tile([HM, 128], f32, tag="U_sb")
                nc.vector.tensor_copy(U_sb[:], ps_U[:])
                ps_UT = ps_mm.tile([128, HM], f32, tag="ct")
                nc.tensor.transpose(ps_UT[:], U_sb[:], ident[:HM, :HM])
                utm = small.tile([128, HM], f32, tag="utm")
                nc.vector.tensor_mul(utm[:], ps_UT[:], blkmask[:])
                concU = small.tile([128, M], f32, tag="concU")
                nc.vector.tensor_reduce(
                    concU[:], utm[:].rearrange("p (h m) -> p m h", h=H),
                    axis=mybir.AxisListType.X, op=add)
                concT = small.tile([128, M], f32, tag="concT")
                nc.vector.tensor_scalar_mul(concT[:], concU[:], zinv[:])

                # ---- final_Q ----
                ps_fq = ps_sm.tile([128, M], f32, tag="sm")
                nc.tensor.matmul(ps_fq[:], poT[:], concT[:])
                fqT = small.tile([128, M], f32, tag="fqT")
                nc.vector.tensor_copy(fqT[:], ps_fq[:])

                # FWl -> block diag lhsT3
                ps_fwl = ps_sm.tile([8, M], f32, tag="sm")
                nc.tensor.matmul(ps_fwl[:], wl8[:], fqT[:])
                fwl = small.tile([8, M], f32, tag="fwls")
                nc.vector.tensor_copy(fwl[:], ps_fwl[:])
                ps_r3 = ps_sm.tile([MF, M], f32, tag="sm")
                nc.tensor.matmul(ps_r3[:], r8[:], fwl[:])
                nc.vector.tensor_mul(lhsT3[0:MF, :], ps_r3[:], maskM[:, 0:M])

                # ---- logits + tanh + mask + exp ----
                eL = mid.tile([M, N], f32, tag="eL")
                rL = small.tile([M, 2], f32, tag="rL")
                for i, (off, ln) in enumerate(HALVES):
                    ps_L = ps_mm.tile([M, 512], f32, tag="mm")
                    nc.tensor.matmul(ps_L[:, :ln], fqT[:], lkst[:, off:off + ln],
                                     start=True, stop=False)
                    nc.tensor.matmul(ps_L[:, :ln], lhsT3[:], ndft[:, off:off + ln],
                                     start=False, stop=True)
                    tl = small.tile([M, 512], f32, tag="tl")
                    nc.scalar.activation(tl[:, :ln], ps_L[:, :ln], TANH,
                                         scale=1.0 / math.sqrt(D))
                    pl = small.tile([M, 512], f32, tag="pl")
                    nc.vector.scalar_tensor_tensor(
                        pl[:, :ln], tl[:, :ln], 10.0,
                        maskb5[:, off:off + ln], op0=mult, op1=add)
                    nc.scalar.activation(eL[:, off:off + ln], pl[:, :ln], EXP,
                                         accum_out=rL[:, i:i + 1])
                rL5 = small.tile([M, 1], f32, tag="rL5")
                nc.vector.tensor_tensor(rL5[:], rL[:, 0:1], rL[:, 1:2], op=add)
                ps_z1 = ps_sm.tile([1, 1], f32, tag="sm")
                nc.tensor.matmul(ps_z1[:], ones5[:], rL5[:])
                z1 = small.tile([1, 1], f32, tag="z1s")
                nc.vector.tensor_copy(z1[:], ps_z1[:])
                zi1 = small.tile([1, 1], f32, tag="zi1")
                nc.vector.reciprocal(zi1[:], z1[:])
                ps_zb = ps_sm.tile([M, 1], f32, tag="sm")
                nc.tensor.matmul(ps_zb[:], ones15[:], zi1[:])
                zb5 = small.tile([M, 1], f32, tag="zb5")
                nc.vector.tensor_copy(zb5[:], ps_zb[:])

                outb = mid.tile([M, N], f32, tag="outb")
                nc.vector.tensor_scalar_mul(outb[:], eL[:], zb5[:])
                nc.sync.dma_start(d_out.ap()[b], outb[:])

    nc.compile()
    return nc


def _prep_inputs(inputs):
    """Host-side shard + relayout (numpy moves only, no arithmetic on data)."""
    gks = inputs["glimpse_K_static"]   # [H, B, 1, N, KS]
    gvs = inputs["glimpse_V_static"]
    lks = inputs["logit_K_static"]     # [B, 1, N, D]
    ndf = inputs["node_dynamic_features"]  # [B, M, N, 8]
    mask = inputs["feasibility_mask"]  # [B, M, N] bool
    prev = inputs["prev_node_embeddings"]  # [B, M, D]
    veh = inputs["vehicle_dynamic_features"]  # [B, M, 3]
    fc = inputs["fixed_context"]       # [B, 1, D]
    W_pcv = inputs["W_pcv"]            # [D, D+3]
    W_pns = inputs["W_pns"]            # [3D, 8]
    po = inputs["po_weight"]           # [D, D]

    f = np.float32
    # [B, 128, NPAD]: row h*16+k = Kstat[h, b, 0, :, k]; zero-padded n
    kst = np.zeros((B, 128, NPAD), dtype=f)
    kst[:, :, :N] = gks[:, :, 0].transpose(1, 0, 3, 2).reshape(B, 128, N)
    lkst = np.ascontiguousarray(lks[:, 0].transpose(0, 2, 1), dtype=f)  # [B,128,N]
    # [B, chunk, np, h*16+k]
    vpad = np.zeros((B, NPAD, 128), dtype=f)
    vpad[:, :N, :] = gvs[:, :, 0].transpose(1, 2, 0, 3).reshape(B, N, 128)
    vst = np.ascontiguousarray(vpad.reshape(B, NCHUNK, 128, 128))
    # ndft: [B, 45, NPAD] rows 0-39 = (m,f), rows 40-44 = mask bias per m
    # (padded n marked infeasible so exp() of padded compat is exactly 0)
    maskb = np.full((B, M, NPAD), MASKVAL, dtype=f)
    maskb[:, :, :N] = np.where(mask, np.float32(0.0), np.float32(MASKVAL))
    ndft = np.zeros((B, MF + M, NPAD), dtype=f)
    ndft[:, :MF, :N] = ndf.transpose(0, 1, 3, 2).reshape(B, MF, N)
    ndft[:, MF:, :] = maskb
    # ndfn: [B, chunk, np, (m,f)+ones] ; ones col counts only real n
    npad = np.zeros((B, NPAD, MF + 1), dtype=f)
    npad[:, :N, :MF] = ndf.transpose(0, 2, 1, 3).reshape(B, N, MF)
    npad[:, :N, MF] = 1.0
    ndfn = np.ascontiguousarray(npad.reshape(B, NCHUNK, 128, MF + 1))
    prevT = np.ascontiguousarray(prev.transpose(0, 2, 1), dtype=f)  # [B,128,M]
    vehT = np.ascontiguousarray(veh.transpose(0, 2, 1), dtype=f)    # [B,3,M]
    fcT = np.ascontiguousarray(fc.transpose(0, 2, 1), dtype=f)      # [B,128,1]

    # constants
    wpcvT = np.ascontiguousarray(W_pcv.T, dtype=f)          # [131, 128]
    wpcvA, wpcvB = wpcvT[:128], wpcvT[128:131]
    wk8 = np.ascontiguousarray(W_pns[128:256], dtype=f)     # [128, 8]
    wl8 = np.ascontiguousarray(W_pns[256:384], dtype=f)     # [128, 8]
    wvstk = np.ascontiguousarray(
        np.tile(W_pns[0:128].T.reshape(1, 8, 128), (M, 1, 1)).reshape(MF, 128),
        dtype=f)                                            # [(m,f), d]
    r8 = np.zeros((8, MF), dtype=f)
    for m in range(M):
        for ff in range(8):
            r8[ff, m * 8 + ff] = 1.0
    maskM = np.zeros((MF, HM), dtype=f)
    for m in range(M):
        for ff in range(8):
            for h in range(H):
                maskM[m * 8 + ff, h * M + m] = 1.0
    mask5 = np.zeros((M, HM), dtype=f)
    for m in range(M):
        for h in range(H):
            mask5[m, h * M + m] = 1.0
    selT = np.zeros((HM, 128), dtype=f)
    for h in range(H):
        for m in range(M):
            selT[h * M + m, h * KS:(h + 1) * KS] = 1.0
    poT = np.ascontiguousarray(po.T, dtype=f)
    ident = np.eye(128, dtype=f)
    ones5 = np.ones((M, 1), dtype=f)
    blkmask = np.zeros((128, HM), dtype=f)
    for h in range(H):
        blkmask[h * KS:(h + 1) * KS, h * M:(h + 1) * M] = 1.0
    ones15 = np.ones((1, M), dtype=f)

    consts = dict(wpcvA=wpcvA, wpcvB=wpcvB, wk8=wk8, wl8=wl8, wvstk=wvstk,
                  r8=r8, maskM=maskM, mask5=mask5, selT=selT, poT=poT,
                  ident=ident, ones5=ones5, ones15=ones15, blkmask=blkmask)

    in_maps = []
    for c in range(NCORES):
        sl = slice(c * BL, (c + 1) * BL)
        m = dict(kst=kst[sl], lkst=lkst[sl], vst=vst[sl], ndft=ndft[sl],
                 ndfn=ndfn[sl], prevT=prevT[sl], vehT=vehT[sl], fc=fcT[sl])
        m.update({k: v.copy() for k, v in consts.items()})
        in_maps.append(m)
    return in_maps


def kernel(**inputs) -> np.ndarray:
    from concourse import bass_utils

    if "nc" not in _CACHE:
        _CACHE["nc"] = _build_program()
    nc = _CACHE["nc"]
    in_maps = _prep_inputs(inputs)
    res = bass_utils.run_bass_kernel_spmd(nc, in_maps, core_ids=list(range(NCORES)))
    outs = [res.results[c]["out"].reshape(BL, M * N) for c in range(NCORES)]
    return np.concatenate(outs, axis=0).astype(np.float32)

